# revision 3
# baseline (speedup 1.0000x reference)
"""TRN2 Bass kernel for nn_CrossAtt: batch-parallel cross-attention over 8 NeuronCores.

Contract: kernel(**inputs) takes the FULL unsharded inputs
  query [8, 2048, 128] f32, x [8, 2048, 128] f32, valid [8, 2048, 2048] i32,
  Wq/Wk/Wv [128, 128] f32
and returns (result [8, 2048, 128] f32, att_w [8, 2048, 2048] f32), matching the
reference  q,k,v = proj(query/x); att_w = softmax(mask(q@k.T)); result = att_w@v.

Sharding: data-parallel over batch — core b computes batch element b, weights
replicated.  Per-core kernel strategy:
  * scores + softmax in natural layout [lq_part, lk_free] so the two big I/O
    tensors (valid in, att_w out) stream contiguously over HBM;
  * softmax max-subtraction is skipped: softmax is shift-invariant and the
    score magnitudes (|att| <~ 30) cannot overflow exp in fp32;
  * masking (x*mask) + row-sum fused in one DVE scalar_tensor_tensor pass
    (int32 mask consumed directly), normalize on ScalarE with per-partition
    reciprocal scale;
  * the output matmul needs att_w with lk on partitions: att_w is cast to bf16
    and block-transposed on-chip by the DMA xbar ([128, 2048] ->
    [128, 16, 128] with out[:, c, :] = column-block c transposed), then
    accumulated against v (bf16) on the PE;
  * score matmuls run as float32r (full-rate fp32 PE mode), projections fp32.
"""

from contextlib import ExitStack

import numpy as np

import concourse.bass as bass
import concourse.tile as tile
from concourse import mybir
from concourse.bass_utils import run_bass_kernel_spmd
from concourse.masks import make_identity

F32 = mybir.dt.float32
F32R = mybir.dt.float32r
BF16 = mybir.dt.bfloat16
I32 = mybir.dt.int32

B = 8
L = 2048
D = 128
P = 128
N_CORES = 8


def _build_cross_att(L: int = L) -> bass.Bass:
    nch = L // P      # 128-row chunks
    nt = L // 512     # 512-wide tiles

    nc = bass.Bass("TRN2", target_bir_lowering=False, debug=False)

    query = nc.dram_tensor("query", [L, D], F32, kind="ExternalInput")
    x = nc.dram_tensor("x", [L, D], F32, kind="ExternalInput")
    valid = nc.dram_tensor("valid", [L, L], I32, kind="ExternalInput")
    Wq = nc.dram_tensor("Wq", [D, D], F32, kind="ExternalInput")
    Wk = nc.dram_tensor("Wk", [D, D], F32, kind="ExternalInput")
    Wv = nc.dram_tensor("Wv", [D, D], F32, kind="ExternalInput")
    att_w = nc.dram_tensor("att_w", [L, L], F32, kind="ExternalOutput")
    result = nc.dram_tensor("result", [L, D], F32, kind="ExternalOutput")

    with tile.TileContext(nc) as tc, ExitStack() as ctx:
        consts = ctx.enter_context(tc.tile_pool(name="consts", bufs=1))

        ident = consts.tile([P, P], F32)
        make_identity(nc, ident[:])

        qT = consts.tile([P, L], F32R)         # [a, lq]
        kT = consts.tile([P, L], F32R)         # [a, lk]
        v_sb = consts.tile([P, nch, D], BF16)  # v_sb[:, c, :] = v[c*128:(c+1)*128, :]

        # ---- projections ----
        with (
            tc.tile_pool(name="pre", bufs=4) as pre,
            tc.tile_pool(name="pre_ps", bufs=2, space="PSUM") as pre_ps,
        ):
            wts = {}
            for name, dram in (("Wq", Wq), ("Wk", Wk), ("Wv", Wv)):
                w_in = pre.tile([P, P], F32, tag="w_in")
                nc.sync.dma_start(w_in[:], dram[:])
                w_ps = pre_ps.tile([P, P], F32, tag="w_ps")
                nc.tensor.transpose(w_ps[:], w_in[:], ident[:])
                wT = consts.tile([P, P], F32R, tag=f"{name}T")
                nc.vector.tensor_copy(wT[:], w_ps[:])
                wts[name] = wT

            qryT = consts.tile([P, L], F32R)
            xT = consts.tile([P, L], F32R)
            for c in range(nch):
                for src, dst in ((query, qryT), (x, xT)):
                    t_in = pre.tile([P, D], F32, tag="t_in")
                    nc.sync.dma_start(t_in[:], src[c * P:(c + 1) * P, :])
                    t_ps = pre_ps.tile([P, P], F32, tag="t_ps")
                    nc.tensor.transpose(t_ps[:], t_in[:], ident[:])
                    nc.vector.tensor_copy(dst[:, c * P:(c + 1) * P], t_ps[:])

            # qT = Wq @ qryT ; kT = Wk @ xT
            for dst, wT, srcT in ((qT, wts["Wq"], qryT), (kT, wts["Wk"], xT)):
                for j in range(nt):
                    sl = slice(j * 512, (j + 1) * 512)
                    p_ps = pre_ps.tile([P, 512], F32, tag="p_ps")
                    nc.tensor.matmul(
                        p_ps[:],
                        wT[:],
                        srcT[:, sl],
                        start=True, stop=True,
                    )
                    nc.scalar.copy(dst[:, sl], p_ps[:])

            # v chunks (natural layout, bf16): v_c = x_c @ Wv.T
            for c in range(nch):
                v_ps = pre_ps.tile([P, D], F32, tag="v_ps")
                nc.tensor.matmul(
                    v_ps[:], xT[:, c * P:(c + 1) * P], wts["Wv"][:],
                    start=True, stop=True,
                )
                nc.scalar.copy(v_sb[:, c, :], v_ps[:])

        # ---- main loop over lq chunks ----
        with (
            tc.tile_pool(name="vload", bufs=3) as vload,
            tc.tile_pool(name="work", bufs=2) as work,
            tc.tile_pool(name="awf32", bufs=3) as awf32,
            tc.tile_pool(name="awT", bufs=2) as awTp,
            tc.tile_pool(name="small", bufs=4) as small,
            tc.tile_pool(name="att_ps", bufs=2, space="PSUM") as att_psp,
            tc.tile_pool(name="res_ps", bufs=2, space="PSUM") as res_psp,
        ):
            for c in range(nch):
                row = slice(c * P, (c + 1) * P)

                valid_sb = vload.tile([P, L], I32)
                nc.sync.dma_start(valid_sb[:], valid[row, :])

                num = work.tile([P, L], F32, tag="num")
                half = L // 2
                for h in range(2):
                    att_ps = att_psp.tile([P, half], F32, tag="att")
                    for j in range(0, half, 512):
                        w = min(512, half - j)
                        nc.tensor.matmul(
                            att_ps[:, j:j + w],
                            qT[:, row],
                            kT[:, h * half + j:h * half + j + w],
                            start=True, stop=True,
                        )
                    nc.scalar.activation(
                        num[:, h * half:(h + 1) * half], att_ps[:],
                        mybir.ActivationFunctionType.Exp,
                    )

                # mask + row sums in one DVE pass (int32 mask used directly)
                num_m = work.tile([P, L], F32, tag="num_m")
                sums = small.tile([P, 1], F32, tag="sums")
                nc.vector.scalar_tensor_tensor(
                    out=num_m[:], in0=num[:], scalar=1.0, in1=valid_sb[:],
                    op0=mybir.AluOpType.mult, op1=mybir.AluOpType.mult,
                    accum_out=sums[:],
                )
                recip = small.tile([P, 1], F32, tag="recip")
                nc.vector.reciprocal(recip[:], sums[:])

                aw = awf32.tile([P, L], F32, tag="aw")
                nc.scalar.mul(aw[:], num_m[:], recip[:])
                nc.sync.dma_start(att_w[row, :], aw[:])

                aw16 = work.tile([P, L], BF16, tag="aw16")
                nc.gpsimd.tensor_copy(aw16[:], aw[:])

                awT = awTp.tile([P, nch, P], BF16, tag="awT")
                nc.scalar.dma_start(awT[:], aw16[:], transpose=True)

                res_ps = res_psp.tile([P, D], F32, tag="res")
                for t in range(nch):
                    nc.tensor.matmul(
                        res_ps[:], awT[:, t, :], v_sb[:, t, :],
                        start=(t == 0), stop=(t == nch - 1),
                    )
                res_sb = small.tile([P, D], F32, tag="res_sb")
                nc.vector.tensor_copy(res_sb[:], res_ps[:])
                nc.scalar.dma_start(result[row, :], res_sb[:])

    return nc


def _split_multi_waits(nc: bass.Bass, max_waits: int = 1) -> int:
    """This walrus build encodes at most one sync-wait per ISA instruction
    ("Too many sync wait commands" otherwise).  Hoist extra waits onto
    standalone EventSemaphore instructions inserted just before, on the same
    engine — the engine stalls on those first, preserving ordering."""
    n_split = 0
    for fn in nc.m.functions:
        for blk in fn.blocks:
            out = []
            changed = False
            for inst in blk.instructions:
                si = inst.sync_info
                waits = list(si.on_wait) if si and si.on_wait else []
                if len(waits) > max_waits and inst.engine is not None:
                    changed = True
                    for j, w in enumerate(waits[:-max_waits]):
                        ev = mybir.InstEventSemaphore(
                            name=f"{inst.name}-sw{j}",
                            engine=inst.engine,
                            ins=[], outs=[],
                        )
                        ev.sync_info = mybir.SyncInfo(on_wait=[w], on_update=[])
                        out.append(ev)
                        n_split += 1
                    inst.sync_info = mybir.SyncInfo(
                        on_wait=waits[-max_waits:],
                        on_update=list(si.on_update) if si.on_update else [],
                    )
                out.append(inst)
            if changed:
                blk.instructions = out
    return n_split


_NC_CACHE: list = []


def _get_nc() -> bass.Bass:
    if not _NC_CACHE:
        nc = _build_cross_att()
        _split_multi_waits(nc)
        _NC_CACHE.append(nc)
    return _NC_CACHE[0]


def kernel(query, x, valid, Wq, Wk, Wv, _trace=False, **_trace_kwargs):
    query = np.ascontiguousarray(query, dtype=np.float32)
    x = np.ascontiguousarray(x, dtype=np.float32)
    valid = np.ascontiguousarray(valid, dtype=np.int32)
    Wq = np.ascontiguousarray(Wq, dtype=np.float32)
    Wk = np.ascontiguousarray(Wk, dtype=np.float32)
    Wv = np.ascontiguousarray(Wv, dtype=np.float32)

    nc = _get_nc()
    in_maps = [
        {"query": query[b], "x": x[b], "valid": valid[b],
         "Wq": Wq, "Wk": Wk, "Wv": Wv}
        for b in range(B)
    ]
    r = run_bass_kernel_spmd(
        nc, in_maps, list(range(N_CORES)), trace=_trace, **_trace_kwargs
    )
    result = np.stack([r.results[b]["result"] for b in range(B)])
    att_w = np.stack([r.results[b]["att_w"] for b in range(B)])
    if _trace:
        return (result, att_w), r
    return result, att_w
